# revision 1
# baseline (speedup 1.0000x reference)
"""Trainium2 Bass kernel for a dense transformer attention layer.

Reference computation (per batch b):
    q,k,v = inp @ w{q,k,v}.T split into 8 heads of 64
    attn  = softmax(q k^T / 8)            [B, H, L, L]  (output 1)
    ctx   = attn @ v
    hid   = LN(ctx + inp)
    out   = LN(relu(hid@w1.T+b1)@w2.T + b2 + hid)       (output 0)

Sharding: 8 cores, core c handles batch c//2 and query rows
[(c%2)*1024, (c%2)*1024+1024). K/V are computed per-core over the full
sequence (replicated across the 2 cores of a batch) so there is no
cross-core communication.
"""
import sys

sys.path.insert(0, "/opt/trn_rl_repo")

import numpy as np

import concourse.bass as bass
import concourse.tile as tile
from concourse import bacc, mybir
from concourse.bass_utils import run_bass_kernel_spmd

F32 = mybir.dt.float32
F32R = mybir.dt.float32r
EXP = mybir.ActivationFunctionType.Exp
SQRT = mybir.ActivationFunctionType.Sqrt
SUB = mybir.AluOpType.subtract
MULT = mybir.AluOpType.mult

B, L, D = 4, 2048, 512
H, DH = 8, 64
LQ = 1024            # query rows per core
NQT = LQ // 128      # 8 query tiles
NKC = L // 128       # 16 key chunks
NDC = D // 128       # 4 contraction chunks
N_CORES = 8
EPS = 1e-6

_CACHE = {}


def _build_program():
    nc = bacc.Bacc("TRN2", target_bir_lowering=False, debug=False,
                   num_devices=N_CORES)

    inpT = nc.dram_tensor("inpT", [D, L], F32, kind="ExternalInput").ap()
    inpTq = nc.dram_tensor("inpTq", [D, LQ], F32, kind="ExternalInput").ap()
    inpN = nc.dram_tensor("inpN", [LQ, D], F32, kind="ExternalInput").ap()
    wqT = nc.dram_tensor("wqT", [D, D], F32, kind="ExternalInput").ap()
    wkT = nc.dram_tensor("wkT", [D, D], F32, kind="ExternalInput").ap()
    wvT = nc.dram_tensor("wvT", [D, D], F32, kind="ExternalInput").ap()
    w1T = nc.dram_tensor("w1T", [D, D], F32, kind="ExternalInput").ap()
    w2T = nc.dram_tensor("w2T", [D, D], F32, kind="ExternalInput").ap()
    vecs = nc.dram_tensor("vecs", [128, 6, D], F32, kind="ExternalInput").ap()
    idn = nc.dram_tensor("idn", [128, 128], F32, kind="ExternalInput").ap()

    attn_o = nc.dram_tensor("attn_o", [H, LQ, L], F32, kind="ExternalOutput").ap()
    out_o = nc.dram_tensor("out_o", [LQ, D], F32, kind="ExternalOutput").ap()

    with tile.TileContext(nc) as tc:
        with tc.tile_pool(name="pc", bufs=1) as pc, \
             tc.tile_pool(name="psS", bufs=2, space="PSUM") as psS, \
             tc.tile_pool(name="psT", bufs=2, space="PSUM") as psT, \
             tc.tile_pool(name="psC", bufs=1, space="PSUM") as psC:

            # ---- persistent SBUF tiles ----
            QT_s = pc.tile([128, NDC, LQ], F32R)     # Q^T (dh-major, /8 folded)
            KT_s = pc.tile([128, NDC, L], F32R)      # K^T (dh-major)
            V_s = pc.tile([128, NKC, D], F32R)       # V natural, per k-chunk
            ctxn_s = pc.tile([128, NQT, D], F32)     # assembled ctx (natural)
            rec_s = pc.tile([128, H * NQT], F32)     # 1/denominator per (h,qt)
            ident = pc.tile([128, 128], F32R)
            eps_t = pc.tile([128, 1], F32)
            vec_s = pc.tile([128, 6, D], F32)        # b1,b2,g1,be1,g2,be2
            w1T_s = pc.tile([128, NDC, D], F32R)
            w2T_s = pc.tile([128, NDC, D], F32R)

            nc.sync.dma_start(out=ident, in_=idn.bitcast(F32R))
            nc.vector.memset(eps_t[:], EPS)
            nc.sync.dma_start(out=vec_s, in_=vecs)
            nc.sync.dma_start(
                out=w1T_s, in_=w1T.rearrange("(c p) m -> p c m", p=128).bitcast(F32R))
            nc.sync.dma_start(
                out=w2T_s, in_=w2T.rearrange("(c p) m -> p c m", p=128).bitcast(F32R))

            # ================= Phase A: projections =================
            with tc.tile_pool(name="pA", bufs=1) as pA:
                inpT_s = pA.tile([128, NDC, L], F32R)
                inpTq_s = pA.tile([128, NDC, LQ], F32R)
                wqT_s = pA.tile([128, NDC, D], F32R)
                wkT_s = pA.tile([128, NDC, D], F32R)
                wvT_s = pA.tile([128, NDC, D], F32R)

                nc.sync.dma_start(
                    out=inpT_s,
                    in_=inpT.rearrange("(c p) l -> p c l", p=128).bitcast(F32R))
                nc.sync.dma_start(
                    out=inpTq_s,
                    in_=inpTq.rearrange("(c p) l -> p c l", p=128).bitcast(F32R))
                for w_s, w_d in ((wqT_s, wqT), (wkT_s, wkT), (wvT_s, wvT)):
                    nc.sync.dma_start(
                        out=w_s,
                        in_=w_d.rearrange("(c p) m -> p c m", p=128).bitcast(F32R))

                # K^T[c*128:(c+1)*128, n*512:(n+1)*512]
                for c in range(NDC):
                    for n in range(L // 512):
                        ps = psT.tile([128, 512], F32, tag="ps512")
                        for dc in range(NDC):
                            nc.tensor.matmul(
                                ps[:],
                                wkT_s[:, dc, c * 128:(c + 1) * 128],
                                inpT_s[:, dc, n * 512:(n + 1) * 512],
                                start=(dc == 0), stop=(dc == NDC - 1))
                        nc.vector.tensor_copy(
                            out=KT_s[:, c, n * 512:(n + 1) * 512], in_=ps[:])
                # Q^T (only the core's query range)
                for c in range(NDC):
                    for n in range(LQ // 512):
                        ps = psT.tile([128, 512], F32, tag="ps512")
                        for dc in range(NDC):
                            nc.tensor.matmul(
                                ps[:],
                                wqT_s[:, dc, c * 128:(c + 1) * 128],
                                inpTq_s[:, dc, n * 512:(n + 1) * 512],
                                start=(dc == 0), stop=(dc == NDC - 1))
                        nc.vector.tensor_copy(
                            out=QT_s[:, c, n * 512:(n + 1) * 512], in_=ps[:])
                # V natural [l, d] per 128-token chunk
                for lc in range(NKC):
                    ps = psT.tile([128, 512], F32, tag="ps512")
                    for dc in range(NDC):
                        nc.tensor.matmul(
                            ps[:],
                            inpT_s[:, dc, lc * 128:(lc + 1) * 128],
                            wvT_s[:, dc, :],
                            start=(dc == 0), stop=(dc == NDC - 1))
                    nc.vector.tensor_copy(out=V_s[:, lc], in_=ps[:])

            # ================= Phase B: attention =================
            with tc.tile_pool(name="pe", bufs=2) as pe, \
                 tc.tile_pool(name="pa", bufs=3) as pa, \
                 tc.tile_pool(name="pt", bufs=4) as ptp, \
                 tc.tile_pool(name="pd", bufs=4) as pd, \
                 tc.tile_pool(name="pct", bufs=2) as pct:

                for h in range(H):
                    hp = (h % 2) * 64
                    hc = h // 2
                    qt_h = QT_s[hp:hp + 64, hc, :]   # [64, LQ]
                    kt_h = KT_s[hp:hp + 64, hc, :]   # [64, L]

                    ctx_ps = psC.tile([64, LQ], F32, tag="ctx")
                    pend = {}

                    def emit_ctx(kc, g):
                        nc.tensor.matmul(
                            ctx_ps[:, g * 512:(g + 1) * 512],
                            V_s[:, kc, h * DH:(h + 1) * DH],
                            pend.pop((kc, g)),
                            start=(kc == 0), stop=(kc == NKC - 1))

                    for qt in range(NQT):
                        # --- S = Q K^T for this q tile (both 1024-col halves)
                        s_half = []
                        for i in range(2):
                            s = psS.tile([128, 1024], F32, tag="s")
                            for j in range(2):
                                n0 = i * 1024 + j * 512
                                nc.tensor.matmul(
                                    s[:, j * 512:(j + 1) * 512],
                                    qt_h[:, qt * 128:(qt + 1) * 128],
                                    kt_h[:, n0:n0 + 512],
                                    start=True, stop=True)
                            s_half.append(s)
                        # --- S^T chunks for ctx (2 k-chunks per q tile),
                        #     software-pipelined one k-chunk behind
                        for kc in (2 * qt, 2 * qt + 1):
                            for g in range(2):
                                st = psT.tile([128, 512], F32, tag="ps512")
                                nc.tensor.matmul(
                                    st[:],
                                    kt_h[:, kc * 128:(kc + 1) * 128],
                                    qt_h[:, g * 512:(g + 1) * 512],
                                    start=True, stop=True)
                                pt = ptp.tile([128, 512], F32R)
                                nc.scalar.activation(out=pt[:], in_=st[:], func=EXP)
                                pend[(kc, g)] = pt
                            if kc > 0:
                                emit_ctx(kc - 1, 0)
                                emit_ctx(kc - 1, 1)
                        # --- exp + row sums + normalize + store attn
                        e_t = pe.tile([128, L], F32)
                        dn = pd.tile([128, 2], F32, tag="dn")
                        for i in range(2):
                            nc.scalar.activation(
                                out=e_t[:, i * 1024:(i + 1) * 1024],
                                in_=s_half[i][:], func=EXP,
                                accum_out=dn[:, i:i + 1])
                        ds = pd.tile([128, 1], F32, tag="ds")
                        nc.vector.tensor_add(ds[:], dn[:, 0:1], dn[:, 1:2])
                        at = pa.tile([128, L], F32)
                        nc.gpsimd.normalize_recip(at[:], e_t[:], ds[:])
                        nc.vector.tensor_copy(
                            out=rec_s[:, h * NQT + qt:h * NQT + qt + 1], in_=ds[:])
                        nc.sync.dma_start(
                            out=attn_o[h, qt * 128:(qt + 1) * 128, :], in_=at[:])
                    emit_ctx(NKC - 1, 0)
                    emit_ctx(NKC - 1, 1)

                    # --- ctx^T -> ctx natural, scaled by 1/denominator
                    ctxT = pct.tile([64, LQ], F32R)
                    nc.vector.tensor_copy(out=ctxT[:], in_=ctx_ps[:])
                    for i in range(NQT):
                        tp = psT.tile([128, DH], F32R, tag="ps512")
                        nc.tensor.transpose(
                            tp[:], ctxT[:, i * 128:(i + 1) * 128],
                            ident[0:64, 0:64])
                        nc.vector.tensor_scalar_mul(
                            out=ctxn_s[:, i, h * DH:(h + 1) * DH],
                            in0=tp[:].bitcast(F32),
                            scalar1=rec_s[:, h * NQT + i:h * NQT + i + 1])

            # ================= Phase C: residual + LN + FFN =================
            b1_s = vec_s[:, 0, :]
            b2_s = vec_s[:, 1, :]
            g1_s = vec_s[:, 2, :]
            be1_s = vec_s[:, 3, :]
            g2_s = vec_s[:, 4, :]
            be2_s = vec_s[:, 5, :]

            def layer_norm(x_ap, g_ap, b_ap, out_tile, spool, fpool):
                st6 = spool.tile([128, 6], F32, tag="st6")
                nc.vector.bn_stats(out=st6[:], in_=x_ap)
                mv = spool.tile([128, 2], F32, tag="mv")
                nc.vector.bn_aggr(out=mv[:], in_=st6[:])
                sd = spool.tile([128, 1], F32, tag="sd")
                nc.scalar.activation(out=sd[:], in_=mv[:, 1:2], func=SQRT,
                                     bias=eps_t[:], scale=1.0)
                rs = spool.tile([128, 1], F32, tag="rs")
                nc.vector.reciprocal(out=rs[:], in_=sd[:])
                nm = fpool.tile([128, D], F32, tag="f")
                nc.vector.tensor_scalar(out=nm[:], in0=x_ap,
                                        scalar1=mv[:, 0:1], scalar2=rs[:],
                                        op0=SUB, op1=MULT)
                gm = fpool.tile([128, D], F32, tag="f")
                nc.vector.tensor_mul(gm[:], nm[:], g_ap)
                nc.vector.tensor_add(out_tile[:], gm[:], b_ap)

            with tc.tile_pool(name="pf", bufs=6) as pf, \
                 tc.tile_pool(name="ph", bufs=2) as ph, \
                 tc.tile_pool(name="pht", bufs=2) as pht, \
                 tc.tile_pool(name="pq", bufs=2) as pq, \
                 tc.tile_pool(name="psm", bufs=8) as psm, \
                 tc.tile_pool(name="po", bufs=2) as po:
                for qt in range(NQT):
                    inq = pq.tile([128, D], F32)
                    nc.sync.dma_start(out=inq,
                                      in_=inpN[qt * 128:(qt + 1) * 128, :])
                    r1 = pf.tile([128, D], F32, tag="f")
                    nc.vector.tensor_add(r1[:], ctxn_s[:, qt], inq[:])
                    hid = ph.tile([128, D], F32R, tag="hid")
                    layer_norm(r1[:], g1_s, be1_s, hid, psm, pf)

                    hT = pht.tile([128, NDC, 128], F32R, tag="ht")
                    for dc in range(NDC):
                        tp = psT.tile([128, 128], F32R, tag="ps512")
                        nc.tensor.transpose(
                            tp[:], hid[:, dc * 128:(dc + 1) * 128], ident[:])
                        nc.vector.tensor_copy(out=hT[:, dc], in_=tp[:])
                    p1 = psT.tile([128, D], F32, tag="ps512")
                    for dc in range(NDC):
                        nc.tensor.matmul(p1[:], hT[:, dc], w1T_s[:, dc],
                                         start=(dc == 0), stop=(dc == NDC - 1))
                    t1 = pf.tile([128, D], F32, tag="f")
                    nc.vector.tensor_add(t1[:], p1[:], b1_s)
                    f1 = ph.tile([128, D], F32R, tag="hid")
                    nc.vector.tensor_scalar_max(out=f1[:], in0=t1[:], scalar1=0.0)

                    fT = pht.tile([128, NDC, 128], F32R, tag="ht")
                    for dc in range(NDC):
                        tp = psT.tile([128, 128], F32R, tag="ps512")
                        nc.tensor.transpose(
                            tp[:], f1[:, dc * 128:(dc + 1) * 128], ident[:])
                        nc.vector.tensor_copy(out=fT[:, dc], in_=tp[:])
                    p2 = psT.tile([128, D], F32, tag="ps512")
                    for dc in range(NDC):
                        nc.tensor.matmul(p2[:], fT[:, dc], w2T_s[:, dc],
                                         start=(dc == 0), stop=(dc == NDC - 1))
                    t2 = pf.tile([128, D], F32, tag="f")
                    nc.vector.tensor_add(t2[:], p2[:], b2_s)
                    r2 = pf.tile([128, D], F32, tag="f")
                    nc.vector.tensor_add(r2[:], t2[:], hid[:].bitcast(F32))
                    o_t = po.tile([128, D], F32)
                    layer_norm(r2[:], g2_s, be2_s, o_t, psm, pf)
                    nc.sync.dma_start(out=out_o[qt * 128:(qt + 1) * 128, :],
                                      in_=o_t[:])

    nc.finalize()
    return nc


def make_in_maps(inp, wq, wk, wv, ln1_g, ln1_b, w1, b1, w2, b2, ln2_g, ln2_b):
    scale = 1.0 / np.sqrt(DH)
    wqT = np.ascontiguousarray((wq * scale).T.astype(np.float32))
    wkT = np.ascontiguousarray(wk.T.astype(np.float32))
    wvT = np.ascontiguousarray(wv.T.astype(np.float32))
    w1T = np.ascontiguousarray(w1.T.astype(np.float32))
    w2T = np.ascontiguousarray(w2.T.astype(np.float32))
    vecs = np.stack([
        np.tile(np.asarray(v, np.float32)[None, :], (128, 1))
        for v in (b1, b2, ln1_g, ln1_b, ln2_g, ln2_b)
    ], axis=1)  # [128, 6, D]
    idn = np.eye(128, dtype=np.float32)

    in_maps = []
    for c in range(N_CORES):
        b = c // 2
        ql = (c % 2) * LQ
        ib = np.asarray(inp[b], np.float32)
        ibT = np.ascontiguousarray(ib.T)
        in_maps.append({
            "inpT": ibT,
            "inpTq": np.ascontiguousarray(ibT[:, ql:ql + LQ]),
            "inpN": np.ascontiguousarray(ib[ql:ql + LQ]),
            "wqT": wqT, "wkT": wkT, "wvT": wvT, "w1T": w1T, "w2T": w2T,
            "vecs": vecs, "idn": idn,
        })
    return in_maps


def kernel(inp, wq, wk, wv, ln1_g, ln1_b, w1, b1, w2, b2, ln2_g, ln2_b):
    if "nc" not in _CACHE:
        _CACHE["nc"] = _build_program()
    nc = _CACHE["nc"]
    in_maps = make_in_maps(inp, wq, wk, wv, ln1_g, ln1_b, w1, b1, w2, b2,
                           ln2_g, ln2_b)
    res = run_bass_kernel_spmd(nc, in_maps, core_ids=list(range(N_CORES)))

    out = np.empty((B, L, D), np.float32)
    attn = np.empty((B, H, L, L), np.float32)
    for c in range(N_CORES):
        b = c // 2
        ql = (c % 2) * LQ
        out[b, ql:ql + LQ] = res.results[c]["out_o"]
        attn[b, :, ql:ql + LQ, :] = res.results[c]["attn_o"]
    return (out, attn)


# revision 3
# speedup vs baseline: 26526.0758x; 26526.0758x over previous
"""Trainium2 Bass kernel for a dense transformer attention layer.

Reference computation (per batch b):
    q,k,v = inp @ w{q,k,v}.T split into 8 heads of 64
    attn  = softmax(q k^T / 8)            [B, H, L, L]  (output 1)
    ctx   = attn @ v
    hid   = LN(ctx + inp)
    out   = LN(relu(hid@w1.T+b1)@w2.T + b2 + hid)       (output 0)

Sharding: 8 cores, core c handles batch c//2 and query rows
[(c%2)*1024, (c%2)*1024+1024). K/V are computed per-core over the full
sequence (replicated across the 2 cores of a batch) so there is no
cross-core communication.
"""
import sys

sys.path.insert(0, "/opt/trn_rl_repo")

import numpy as np

import concourse.bass as bass
import concourse.tile as tile
from concourse import bacc, mybir
from concourse.bass_utils import run_bass_kernel_spmd

F32 = mybir.dt.float32
F32R = mybir.dt.float32r
EXP = mybir.ActivationFunctionType.Exp
SQRT = mybir.ActivationFunctionType.Sqrt
SUB = mybir.AluOpType.subtract
MULT = mybir.AluOpType.mult

B, L, D = 4, 2048, 512
H, DH = 8, 64
LQ = 1024            # query rows per core
NQT = LQ // 128      # 8 query tiles
NKC = L // 128       # 16 key chunks
NDC = D // 128       # 4 contraction chunks
N_CORES = 8
EPS = 1e-6

_CACHE = {}


def _build_program(timing_reps=None):
    nc = bacc.Bacc("TRN2", target_bir_lowering=False, debug=False,
                   num_devices=N_CORES)

    inpT = nc.dram_tensor("inpT", [D, L], F32, kind="ExternalInput").ap()
    inpTq = nc.dram_tensor("inpTq", [D, LQ], F32, kind="ExternalInput").ap()
    inpN = nc.dram_tensor("inpN", [LQ, D], F32, kind="ExternalInput").ap()
    wqT = nc.dram_tensor("wqT", [D, D], F32, kind="ExternalInput").ap()
    wkT = nc.dram_tensor("wkT", [D, D], F32, kind="ExternalInput").ap()
    wvT = nc.dram_tensor("wvT", [D, D], F32, kind="ExternalInput").ap()
    w1T = nc.dram_tensor("w1T", [D, D], F32, kind="ExternalInput").ap()
    w2T = nc.dram_tensor("w2T", [D, D], F32, kind="ExternalInput").ap()
    vecs = nc.dram_tensor("vecs", [128, 6, D], F32, kind="ExternalInput").ap()
    idn = nc.dram_tensor("idn", [128, 128], F32, kind="ExternalInput").ap()

    attn_kind = "Internal" if timing_reps else "ExternalOutput"
    attn_o = nc.dram_tensor("attn_o", [H, LQ, L], F32, kind=attn_kind).ap()
    out_o = nc.dram_tensor("out_o", [LQ, D], F32, kind="ExternalOutput").ap()

    with tile.TileContext(nc) as tc:
        if timing_reps:
            _loop = tc.For_i(0, timing_reps, 1)
            _loop.__enter__()
        with tc.tile_pool(name="pc", bufs=1) as pc, \
             tc.tile_pool(name="psS", bufs=2, space="PSUM") as psS, \
             tc.tile_pool(name="psT", bufs=2, space="PSUM") as psT, \
             tc.tile_pool(name="psC", bufs=1, space="PSUM") as psC:

            # ---- persistent SBUF tiles ----
            QT_s = pc.tile([128, NDC, LQ], F32R)     # Q^T (dh-major, /8 folded)
            KT_s = pc.tile([128, NDC, L], F32R)      # K^T (dh-major)
            V_s = pc.tile([128, NKC, D], F32R)       # V natural, per k-chunk
            ctxn_s = pc.tile([128, NQT, D], F32)     # assembled ctx (natural)
            rec_s = pc.tile([128, H * NQT], F32)     # 1/denominator per (h,qt)
            ident = pc.tile([128, 128], F32R)
            eps_t = pc.tile([128, 1], F32)
            vec_s = pc.tile([128, 6, D], F32)        # b1,b2,g1,be1,g2,be2
            w1T_s = pc.tile([128, NDC, D], F32R)
            w2T_s = pc.tile([128, NDC, D], F32R)

            nc.sync.dma_start(out=ident, in_=idn.bitcast(F32R))
            nc.vector.memset(eps_t[:], EPS)
            nc.sync.dma_start(out=vec_s, in_=vecs)
            nc.sync.dma_start(
                out=w1T_s, in_=w1T.rearrange("(c p) m -> p c m", p=128).bitcast(F32R))
            nc.sync.dma_start(
                out=w2T_s, in_=w2T.rearrange("(c p) m -> p c m", p=128).bitcast(F32R))

            # ================= Phase A: projections =================
            with tc.tile_pool(name="pA", bufs=1) as pA:
                inpT_s = pA.tile([128, NDC, L], F32R)
                inpTq_s = pA.tile([128, NDC, LQ], F32R)
                wqT_s = pA.tile([128, NDC, D], F32R)
                wkT_s = pA.tile([128, NDC, D], F32R)
                wvT_s = pA.tile([128, NDC, D], F32R)

                nc.sync.dma_start(
                    out=inpT_s,
                    in_=inpT.rearrange("(c p) l -> p c l", p=128).bitcast(F32R))
                nc.sync.dma_start(
                    out=inpTq_s,
                    in_=inpTq.rearrange("(c p) l -> p c l", p=128).bitcast(F32R))
                for w_s, w_d in ((wqT_s, wqT), (wkT_s, wkT), (wvT_s, wvT)):
                    nc.sync.dma_start(
                        out=w_s,
                        in_=w_d.rearrange("(c p) m -> p c m", p=128).bitcast(F32R))

                # K^T[c*128:(c+1)*128, n*512:(n+1)*512]
                for c in range(NDC):
                    for n in range(L // 512):
                        ps = psT.tile([128, 512], F32, tag="ps512")
                        for dc in range(NDC):
                            nc.tensor.matmul(
                                ps[:],
                                wkT_s[:, dc, c * 128:(c + 1) * 128],
                                inpT_s[:, dc, n * 512:(n + 1) * 512],
                                start=(dc == 0), stop=(dc == NDC - 1))
                        nc.vector.tensor_copy(
                            out=KT_s[:, c, n * 512:(n + 1) * 512], in_=ps[:])
                # Q^T (only the core's query range)
                for c in range(NDC):
                    for n in range(LQ // 512):
                        ps = psT.tile([128, 512], F32, tag="ps512")
                        for dc in range(NDC):
                            nc.tensor.matmul(
                                ps[:],
                                wqT_s[:, dc, c * 128:(c + 1) * 128],
                                inpTq_s[:, dc, n * 512:(n + 1) * 512],
                                start=(dc == 0), stop=(dc == NDC - 1))
                        nc.vector.tensor_copy(
                            out=QT_s[:, c, n * 512:(n + 1) * 512], in_=ps[:])
                # V natural [l, d] per 128-token chunk
                for lc in range(NKC):
                    ps = psT.tile([128, 512], F32, tag="ps512")
                    for dc in range(NDC):
                        nc.tensor.matmul(
                            ps[:],
                            inpT_s[:, dc, lc * 128:(lc + 1) * 128],
                            wvT_s[:, dc, :],
                            start=(dc == 0), stop=(dc == NDC - 1))
                    nc.vector.tensor_copy(out=V_s[:, lc], in_=ps[:])

            # ================= Phase B: attention =================
            with tc.tile_pool(name="pe", bufs=2) as pe, \
                 tc.tile_pool(name="pa", bufs=3) as pa, \
                 tc.tile_pool(name="pt", bufs=4) as ptp, \
                 tc.tile_pool(name="pd", bufs=4) as pd, \
                 tc.tile_pool(name="pct", bufs=2) as pct:

                for h in range(H):
                    hp = (h % 2) * 64
                    hc = h // 2
                    qt_h = QT_s[hp:hp + 64, hc, :]   # [64, LQ]
                    kt_h = KT_s[hp:hp + 64, hc, :]   # [64, L]

                    ctx_ps = psC.tile([64, LQ], F32, tag="ctx")
                    pend = {}

                    def emit_ctx(kc, g):
                        nc.tensor.matmul(
                            ctx_ps[:, g * 512:(g + 1) * 512],
                            V_s[:, kc, h * DH:(h + 1) * DH],
                            pend.pop((kc, g)),
                            start=(kc == 0), stop=(kc == NKC - 1))

                    for qt in range(NQT):
                        # --- S = Q K^T for this q tile (both 1024-col halves)
                        s_half = []
                        for i in range(2):
                            s = psS.tile([128, 1024], F32, tag="s")
                            for j in range(2):
                                n0 = i * 1024 + j * 512
                                nc.tensor.matmul(
                                    s[:, j * 512:(j + 1) * 512],
                                    qt_h[:, qt * 128:(qt + 1) * 128],
                                    kt_h[:, n0:n0 + 512],
                                    start=True, stop=True)
                            s_half.append(s)
                        # --- S^T chunks for ctx (2 k-chunks per q tile),
                        #     software-pipelined one k-chunk behind
                        for kc in (2 * qt, 2 * qt + 1):
                            for g in range(2):
                                st = psT.tile([128, 512], F32, tag="ps512")
                                nc.tensor.matmul(
                                    st[:],
                                    kt_h[:, kc * 128:(kc + 1) * 128],
                                    qt_h[:, g * 512:(g + 1) * 512],
                                    start=True, stop=True)
                                pt = ptp.tile([128, 512], F32R)
                                nc.scalar.activation(out=pt[:], in_=st[:], func=EXP)
                                pend[(kc, g)] = pt
                            if kc > 0:
                                emit_ctx(kc - 1, 0)
                                emit_ctx(kc - 1, 1)
                        # --- exp + row sums + normalize + store attn
                        e_t = pe.tile([128, L], F32)
                        dn = pd.tile([128, 2], F32, tag="dn")
                        for i in range(2):
                            nc.scalar.activation(
                                out=e_t[:, i * 1024:(i + 1) * 1024],
                                in_=s_half[i][:], func=EXP,
                                accum_out=dn[:, i:i + 1])
                        ds = pd.tile([128, 1], F32, tag="ds")
                        nc.vector.tensor_add(ds[:], dn[:, 0:1], dn[:, 1:2])
                        at = pa.tile([128, L], F32)
                        nc.gpsimd.normalize_recip(at[:], e_t[:], ds[:])
                        nc.vector.tensor_copy(
                            out=rec_s[:, h * NQT + qt:h * NQT + qt + 1], in_=ds[:])
                        nc.sync.dma_start(
                            out=attn_o[h, qt * 128:(qt + 1) * 128, :], in_=at[:])
                    emit_ctx(NKC - 1, 0)
                    emit_ctx(NKC - 1, 1)

                    # --- ctx^T -> ctx natural, scaled by 1/denominator
                    ctxT = pct.tile([64, LQ], F32R)
                    nc.vector.tensor_copy(out=ctxT[:], in_=ctx_ps[:])
                    for i in range(NQT):
                        tp = psT.tile([128, DH], F32R, tag="ps512")
                        nc.tensor.transpose(
                            tp[:], ctxT[:, i * 128:(i + 1) * 128],
                            ident[0:64, 0:64])
                        nc.vector.tensor_scalar_mul(
                            out=ctxn_s[:, i, h * DH:(h + 1) * DH],
                            in0=tp[:].bitcast(F32),
                            scalar1=rec_s[:, h * NQT + i:h * NQT + i + 1])

            # ================= Phase C: residual + LN + FFN =================
            b1_s = vec_s[:, 0, :]
            b2_s = vec_s[:, 1, :]
            g1_s = vec_s[:, 2, :]
            be1_s = vec_s[:, 3, :]
            g2_s = vec_s[:, 4, :]
            be2_s = vec_s[:, 5, :]

            def layer_norm(x_ap, g_ap, b_ap, out_tile, spool, fpool):
                st6 = spool.tile([128, 6], F32, tag="st6")
                nc.vector.bn_stats(out=st6[:], in_=x_ap)
                mv = spool.tile([128, 2], F32, tag="mv")
                nc.vector.bn_aggr(out=mv[:], in_=st6[:])
                sd = spool.tile([128, 1], F32, tag="sd")
                nc.scalar.activation(out=sd[:], in_=mv[:, 1:2], func=SQRT,
                                     bias=eps_t[:], scale=1.0)
                rs = spool.tile([128, 1], F32, tag="rs")
                nc.vector.reciprocal(out=rs[:], in_=sd[:])
                nm = fpool.tile([128, D], F32, tag="f")
                nc.vector.tensor_scalar(out=nm[:], in0=x_ap,
                                        scalar1=mv[:, 0:1], scalar2=rs[:],
                                        op0=SUB, op1=MULT)
                gm = fpool.tile([128, D], F32, tag="f")
                nc.vector.tensor_mul(gm[:], nm[:], g_ap)
                nc.vector.tensor_add(out_tile[:], gm[:], b_ap)

            with tc.tile_pool(name="pf", bufs=6) as pf, \
                 tc.tile_pool(name="ph", bufs=2) as ph, \
                 tc.tile_pool(name="pht", bufs=2) as pht, \
                 tc.tile_pool(name="pq", bufs=2) as pq, \
                 tc.tile_pool(name="psm", bufs=8) as psm, \
                 tc.tile_pool(name="po", bufs=2) as po:
                for qt in range(NQT):
                    inq = pq.tile([128, D], F32)
                    nc.sync.dma_start(out=inq,
                                      in_=inpN[qt * 128:(qt + 1) * 128, :])
                    r1 = pf.tile([128, D], F32, tag="f")
                    nc.vector.tensor_add(r1[:], ctxn_s[:, qt], inq[:])
                    hid = ph.tile([128, D], F32R, tag="hid")
                    layer_norm(r1[:], g1_s, be1_s, hid, psm, pf)

                    hT = pht.tile([128, NDC, 128], F32R, tag="ht")
                    for dc in range(NDC):
                        tp = psT.tile([128, 128], F32R, tag="ps512")
                        nc.tensor.transpose(
                            tp[:], hid[:, dc * 128:(dc + 1) * 128], ident[:])
                        nc.vector.tensor_copy(out=hT[:, dc], in_=tp[:])
                    p1 = psT.tile([128, D], F32, tag="ps512")
                    for dc in range(NDC):
                        nc.tensor.matmul(p1[:], hT[:, dc], w1T_s[:, dc],
                                         start=(dc == 0), stop=(dc == NDC - 1))
                    t1 = pf.tile([128, D], F32, tag="f")
                    nc.vector.tensor_add(t1[:], p1[:], b1_s)
                    f1 = ph.tile([128, D], F32R, tag="hid")
                    nc.vector.tensor_scalar_max(out=f1[:], in0=t1[:], scalar1=0.0)

                    fT = pht.tile([128, NDC, 128], F32R, tag="ht")
                    for dc in range(NDC):
                        tp = psT.tile([128, 128], F32R, tag="ps512")
                        nc.tensor.transpose(
                            tp[:], f1[:, dc * 128:(dc + 1) * 128], ident[:])
                        nc.vector.tensor_copy(out=fT[:, dc], in_=tp[:])
                    p2 = psT.tile([128, D], F32, tag="ps512")
                    for dc in range(NDC):
                        nc.tensor.matmul(p2[:], fT[:, dc], w2T_s[:, dc],
                                         start=(dc == 0), stop=(dc == NDC - 1))
                    t2 = pf.tile([128, D], F32, tag="f")
                    nc.vector.tensor_add(t2[:], p2[:], b2_s)
                    r2 = pf.tile([128, D], F32, tag="f")
                    nc.vector.tensor_add(r2[:], t2[:], hid[:].bitcast(F32))
                    o_t = po.tile([128, D], F32)
                    layer_norm(r2[:], g2_s, be2_s, o_t, psm, pf)
                    nc.sync.dma_start(out=out_o[qt * 128:(qt + 1) * 128, :],
                                      in_=o_t[:])
        if timing_reps:
            _loop.__exit__(None, None, None)

    nc.finalize()
    return nc


def make_in_maps(inp, wq, wk, wv, ln1_g, ln1_b, w1, b1, w2, b2, ln2_g, ln2_b):
    scale = 1.0 / np.sqrt(DH)
    wqT = np.ascontiguousarray((wq * scale).T.astype(np.float32))
    wkT = np.ascontiguousarray(wk.T.astype(np.float32))
    wvT = np.ascontiguousarray(wv.T.astype(np.float32))
    w1T = np.ascontiguousarray(w1.T.astype(np.float32))
    w2T = np.ascontiguousarray(w2.T.astype(np.float32))
    vecs = np.stack([
        np.tile(np.asarray(v, np.float32)[None, :], (128, 1))
        for v in (b1, b2, ln1_g, ln1_b, ln2_g, ln2_b)
    ], axis=1)  # [128, 6, D]
    idn = np.eye(128, dtype=np.float32)

    in_maps = []
    for c in range(N_CORES):
        b = c // 2
        ql = (c % 2) * LQ
        ib = np.asarray(inp[b], np.float32)
        ibT = np.ascontiguousarray(ib.T)
        in_maps.append({
            "inpT": ibT,
            "inpTq": np.ascontiguousarray(ibT[:, ql:ql + LQ]),
            "inpN": np.ascontiguousarray(ib[ql:ql + LQ]),
            "wqT": wqT, "wkT": wkT, "wvT": wvT, "w1T": w1T, "w2T": w2T,
            "vecs": vecs, "idn": idn,
        })
    return in_maps


def kernel(inp, wq, wk, wv, ln1_g, ln1_b, w1, b1, w2, b2, ln2_g, ln2_b):
    if "nc" not in _CACHE:
        _CACHE["nc"] = _build_program()
    nc = _CACHE["nc"]
    in_maps = make_in_maps(inp, wq, wk, wv, ln1_g, ln1_b, w1, b1, w2, b2,
                           ln2_g, ln2_b)
    res = run_bass_kernel_spmd(nc, in_maps, core_ids=list(range(N_CORES)))

    out = np.empty((B, L, D), np.float32)
    attn = np.empty((B, H, L, L), np.float32)
    for c in range(N_CORES):
        b = c // 2
        ql = (c % 2) * LQ
        out[b, ql:ql + LQ] = res.results[c]["out_o"]
        attn[b, :, ql:ql + LQ, :] = res.results[c]["attn_o"]
    return (out, attn)


# revision 5
# speedup vs baseline: 32477.4750x; 1.2244x over previous
"""Trainium2 Bass kernel for a dense transformer attention layer.

Reference computation (per batch b):
    q,k,v = inp @ w{q,k,v}.T split into 8 heads of 64
    attn  = softmax(q k^T / 8)            [B, H, L, L]  (output 1)
    ctx   = attn @ v
    hid   = LN(ctx + inp)
    out   = LN(relu(hid@w1.T+b1)@w2.T + b2 + hid)       (output 0)

Sharding: 8 cores, core c handles batch c//2 and query rows
[(c%2)*1024, (c%2)*1024+1024). K/V are computed per-core over the full
sequence (replicated across the 2 cores of a batch) so there is no
cross-core communication.
"""
import sys

sys.path.insert(0, "/opt/trn_rl_repo")

import numpy as np

import concourse.bass as bass
import concourse.tile as tile
from concourse import bacc, mybir
from concourse.bass_utils import run_bass_kernel_spmd

F32 = mybir.dt.float32
F32R = mybir.dt.float32r
EXP = mybir.ActivationFunctionType.Exp
SQRT = mybir.ActivationFunctionType.Sqrt
SUB = mybir.AluOpType.subtract
MULT = mybir.AluOpType.mult

B, L, D = 4, 2048, 512
H, DH = 8, 64
LQ = 1024            # query rows per core
NQT = LQ // 128      # 8 query tiles
NKC = L // 128       # 16 key chunks
NDC = D // 128       # 4 contraction chunks
N_CORES = 8
EPS = 1e-6

_CACHE = {}


def _build_program(timing_reps=None, variant="full"):
    skip_attn_dma = variant in ("noattndma", "noqk")
    skip_qk = variant == "noqk"
    skip_norm = variant in ("nonorm", "noqk")
    dve_exp = variant == "dveexp"
    nc = bacc.Bacc("TRN2", target_bir_lowering=False, debug=False,
                   num_devices=N_CORES)

    inpT = nc.dram_tensor("inpT", [D, L], F32, kind="ExternalInput").ap()
    inpTq = nc.dram_tensor("inpTq", [D, LQ], F32, kind="ExternalInput").ap()
    inpN = nc.dram_tensor("inpN", [LQ, D], F32, kind="ExternalInput").ap()
    wqT = nc.dram_tensor("wqT", [D, D], F32, kind="ExternalInput").ap()
    wkT = nc.dram_tensor("wkT", [D, D], F32, kind="ExternalInput").ap()
    wvT = nc.dram_tensor("wvT", [D, D], F32, kind="ExternalInput").ap()
    w1T = nc.dram_tensor("w1T", [D, D], F32, kind="ExternalInput").ap()
    w2T = nc.dram_tensor("w2T", [D, D], F32, kind="ExternalInput").ap()
    vecs = nc.dram_tensor("vecs", [128, 6, D], F32, kind="ExternalInput").ap()
    idn = nc.dram_tensor("idn", [128, 128], F32, kind="ExternalInput").ap()

    attn_kind = "Internal" if timing_reps else "ExternalOutput"
    attn_o = nc.dram_tensor("attn_o", [H, LQ, L], F32, kind=attn_kind).ap()
    out_o = nc.dram_tensor("out_o", [LQ, D], F32, kind="ExternalOutput").ap()

    with tile.TileContext(nc) as tc:
        if timing_reps:
            _loop = tc.For_i(0, timing_reps, 1)
            _loop.__enter__()
        with tc.tile_pool(name="pc", bufs=1) as pc, \
             tc.tile_pool(name="psS", bufs=2, space="PSUM") as psS, \
             tc.tile_pool(name="psT", bufs=2, space="PSUM") as psT, \
             tc.tile_pool(name="psC", bufs=1, space="PSUM") as psC:

            # ---- persistent SBUF tiles ----
            QT_s = pc.tile([128, NDC, LQ], F32R)     # Q^T (dh-major, /8 folded)
            KT_s = pc.tile([128, NDC, L], F32R)      # K^T (dh-major)
            V_s = pc.tile([128, NKC, D], F32R)       # V natural, per k-chunk
            ctxn_s = pc.tile([128, NQT, D], F32)     # assembled ctx (natural)
            rec_s = pc.tile([128, H * NQT], F32)     # 1/denominator per (h,qt)
            ident = pc.tile([128, 128], F32R)
            eps_t = pc.tile([128, 1], F32)
            vec_s = pc.tile([128, 6, D], F32)        # b1,b2,g1,be1,g2,be2
            w1T_s = pc.tile([128, NDC, D], F32R)
            w2T_s = pc.tile([128, NDC, D], F32R)

            nc.sync.dma_start(out=ident, in_=idn.bitcast(F32R))
            nc.vector.memset(eps_t[:], EPS)
            nc.sync.dma_start(out=vec_s, in_=vecs)
            nc.sync.dma_start(
                out=w1T_s, in_=w1T.rearrange("(c p) m -> p c m", p=128).bitcast(F32R))
            nc.sync.dma_start(
                out=w2T_s, in_=w2T.rearrange("(c p) m -> p c m", p=128).bitcast(F32R))

            # ================= Phase A: projections =================
            with tc.tile_pool(name="pA", bufs=1) as pA:
                inpT_s = pA.tile([128, NDC, L], F32R)
                inpTq_s = pA.tile([128, NDC, LQ], F32R)
                wqT_s = pA.tile([128, NDC, D], F32R)
                wkT_s = pA.tile([128, NDC, D], F32R)
                wvT_s = pA.tile([128, NDC, D], F32R)

                nc.sync.dma_start(
                    out=inpT_s,
                    in_=inpT.rearrange("(c p) l -> p c l", p=128).bitcast(F32R))
                nc.sync.dma_start(
                    out=inpTq_s,
                    in_=inpTq.rearrange("(c p) l -> p c l", p=128).bitcast(F32R))
                for w_s, w_d in ((wqT_s, wqT), (wkT_s, wkT), (wvT_s, wvT)):
                    nc.sync.dma_start(
                        out=w_s,
                        in_=w_d.rearrange("(c p) m -> p c m", p=128).bitcast(F32R))

                # K^T[c*128:(c+1)*128, n*512:(n+1)*512]
                for c in range(NDC):
                    for n in range(L // 512):
                        ps = psT.tile([128, 512], F32, tag="ps512")
                        for dc in range(NDC):
                            nc.tensor.matmul(
                                ps[:],
                                wkT_s[:, dc, c * 128:(c + 1) * 128],
                                inpT_s[:, dc, n * 512:(n + 1) * 512],
                                start=(dc == 0), stop=(dc == NDC - 1))
                        nc.vector.tensor_copy(
                            out=KT_s[:, c, n * 512:(n + 1) * 512], in_=ps[:])
                # Q^T (only the core's query range)
                for c in range(NDC):
                    for n in range(LQ // 512):
                        ps = psT.tile([128, 512], F32, tag="ps512")
                        for dc in range(NDC):
                            nc.tensor.matmul(
                                ps[:],
                                wqT_s[:, dc, c * 128:(c + 1) * 128],
                                inpTq_s[:, dc, n * 512:(n + 1) * 512],
                                start=(dc == 0), stop=(dc == NDC - 1))
                        nc.vector.tensor_copy(
                            out=QT_s[:, c, n * 512:(n + 1) * 512], in_=ps[:])
                # V natural [l, d] per 128-token chunk
                for lc in range(NKC):
                    ps = psT.tile([128, 512], F32, tag="ps512")
                    for dc in range(NDC):
                        nc.tensor.matmul(
                            ps[:],
                            inpT_s[:, dc, lc * 128:(lc + 1) * 128],
                            wvT_s[:, dc, :],
                            start=(dc == 0), stop=(dc == NDC - 1))
                    nc.vector.tensor_copy(out=V_s[:, lc], in_=ps[:])

            # ================= Phase B: attention =================
            with tc.tile_pool(name="pe", bufs=2) as pe, \
                 tc.tile_pool(name="pa", bufs=3) as pa, \
                 tc.tile_pool(name="pt", bufs=4) as ptp, \
                 tc.tile_pool(name="pd", bufs=4) as pd, \
                 tc.tile_pool(name="pct", bufs=2) as pct:

                for h in range(H):
                    hp = (h % 2) * 64
                    hc = h // 2
                    qt_h = QT_s[hp:hp + 64, hc, :]   # [64, LQ]
                    kt_h = KT_s[hp:hp + 64, hc, :]   # [64, L]

                    ctx_ps = psC.tile([64, LQ], F32, tag="ctx")
                    pend = {}

                    def emit_ctx(kc, g):
                        nc.tensor.matmul(
                            ctx_ps[:, g * 512:(g + 1) * 512],
                            V_s[:, kc, h * DH:(h + 1) * DH],
                            pend.pop((kc, g)),
                            start=(kc == 0), stop=(kc == NKC - 1))

                    for qt in range(NQT):
                        # --- S = Q K^T for this q tile (both 1024-col halves)
                        s_half = []
                        if not skip_qk:
                            for i in range(2):
                                s = psS.tile([128, 1024], F32, tag="s")
                                for j in range(2):
                                    n0 = i * 1024 + j * 512
                                    nc.tensor.matmul(
                                        s[:, j * 512:(j + 1) * 512],
                                        qt_h[:, qt * 128:(qt + 1) * 128],
                                        kt_h[:, n0:n0 + 512],
                                        start=True, stop=True)
                                s_half.append(s)
                        # --- S^T chunks for ctx (2 k-chunks per q tile),
                        #     software-pipelined one k-chunk behind
                        for kc in (2 * qt, 2 * qt + 1):
                            for g in range(2):
                                st = psT.tile([128, 512], F32, tag="ps512")
                                nc.tensor.matmul(
                                    st[:],
                                    kt_h[:, kc * 128:(kc + 1) * 128],
                                    qt_h[:, g * 512:(g + 1) * 512],
                                    start=True, stop=True)
                                pt = ptp.tile([128, 512], F32R)
                                if dve_exp:
                                    nc.vector.tensor_copy(out=pt[:], in_=st[:])
                                else:
                                    nc.scalar.activation(out=pt[:], in_=st[:],
                                                         func=EXP)
                                pend[(kc, g)] = pt
                            if kc > 0:
                                emit_ctx(kc - 1, 0)
                                emit_ctx(kc - 1, 1)
                        if skip_qk:
                            continue
                        # --- exp + row sums + normalize + store attn
                        e_t = pe.tile([128, L], F32)
                        dn = pd.tile([128, 2], F32, tag="dn")
                        for i in range(2):
                            nc.scalar.activation(
                                out=e_t[:, i * 1024:(i + 1) * 1024],
                                in_=s_half[i][:], func=EXP,
                                accum_out=dn[:, i:i + 1])
                        ds = pd.tile([128, 1], F32, tag="ds")
                        nc.vector.tensor_add(ds[:], dn[:, 0:1], dn[:, 1:2])
                        at = pa.tile([128, L], F32)
                        if skip_norm:
                            nc.vector.reciprocal(out=ds[:], in_=ds[:])
                            at = e_t
                        else:
                            nc.gpsimd.normalize_recip(at[:], e_t[:], ds[:])
                        nc.vector.tensor_copy(
                            out=rec_s[:, h * NQT + qt:h * NQT + qt + 1], in_=ds[:])
                        if not skip_attn_dma:
                            nc.sync.dma_start(
                                out=attn_o[h, qt * 128:(qt + 1) * 128, :], in_=at[:])
                    emit_ctx(NKC - 1, 0)
                    emit_ctx(NKC - 1, 1)

                    # --- ctx^T -> ctx natural, scaled by 1/denominator
                    ctxT = pct.tile([64, LQ], F32R)
                    nc.vector.tensor_copy(out=ctxT[:], in_=ctx_ps[:])
                    for i in range(NQT):
                        tp = psT.tile([128, DH], F32R, tag="ps512")
                        nc.tensor.transpose(
                            tp[:], ctxT[:, i * 128:(i + 1) * 128],
                            ident[0:64, 0:64])
                        nc.vector.tensor_scalar_mul(
                            out=ctxn_s[:, i, h * DH:(h + 1) * DH],
                            in0=tp[:].bitcast(F32),
                            scalar1=rec_s[:, h * NQT + i:h * NQT + i + 1])

            # ================= Phase C: residual + LN + FFN =================
            b1_s = vec_s[:, 0, :]
            b2_s = vec_s[:, 1, :]
            g1_s = vec_s[:, 2, :]
            be1_s = vec_s[:, 3, :]
            g2_s = vec_s[:, 4, :]
            be2_s = vec_s[:, 5, :]

            def layer_norm(x_ap, g_ap, b_ap, out_tile, spool, fpool):
                st6 = spool.tile([128, 6], F32, tag="st6")
                nc.vector.bn_stats(out=st6[:], in_=x_ap)
                mv = spool.tile([128, 2], F32, tag="mv")
                nc.vector.bn_aggr(out=mv[:], in_=st6[:])
                sd = spool.tile([128, 1], F32, tag="sd")
                nc.scalar.activation(out=sd[:], in_=mv[:, 1:2], func=SQRT,
                                     bias=eps_t[:], scale=1.0)
                rs = spool.tile([128, 1], F32, tag="rs")
                nc.vector.reciprocal(out=rs[:], in_=sd[:])
                nm = fpool.tile([128, D], F32, tag="f")
                nc.vector.tensor_scalar(out=nm[:], in0=x_ap,
                                        scalar1=mv[:, 0:1], scalar2=rs[:],
                                        op0=SUB, op1=MULT)
                gm = fpool.tile([128, D], F32, tag="f")
                nc.vector.tensor_mul(gm[:], nm[:], g_ap)
                nc.vector.tensor_add(out_tile[:], gm[:], b_ap)

            with tc.tile_pool(name="pf", bufs=6) as pf, \
                 tc.tile_pool(name="ph", bufs=2) as ph, \
                 tc.tile_pool(name="pht", bufs=2) as pht, \
                 tc.tile_pool(name="pq", bufs=2) as pq, \
                 tc.tile_pool(name="psm", bufs=8) as psm, \
                 tc.tile_pool(name="po", bufs=2) as po:
                for qt in range(NQT):
                    inq = pq.tile([128, D], F32)
                    nc.sync.dma_start(out=inq,
                                      in_=inpN[qt * 128:(qt + 1) * 128, :])
                    r1 = pf.tile([128, D], F32, tag="f")
                    nc.vector.tensor_add(r1[:], ctxn_s[:, qt], inq[:])
                    hid = ph.tile([128, D], F32R, tag="hid")
                    layer_norm(r1[:], g1_s, be1_s, hid, psm, pf)

                    hT = pht.tile([128, NDC, 128], F32R, tag="ht")
                    for dc in range(NDC):
                        tp = psT.tile([128, 128], F32R, tag="ps512")
                        nc.tensor.transpose(
                            tp[:], hid[:, dc * 128:(dc + 1) * 128], ident[:])
                        nc.vector.tensor_copy(out=hT[:, dc], in_=tp[:])
                    p1 = psT.tile([128, D], F32, tag="ps512")
                    for dc in range(NDC):
                        nc.tensor.matmul(p1[:], hT[:, dc], w1T_s[:, dc],
                                         start=(dc == 0), stop=(dc == NDC - 1))
                    t1 = pf.tile([128, D], F32, tag="f")
                    nc.vector.tensor_add(t1[:], p1[:], b1_s)
                    f1 = ph.tile([128, D], F32R, tag="hid")
                    nc.vector.tensor_scalar_max(out=f1[:], in0=t1[:], scalar1=0.0)

                    fT = pht.tile([128, NDC, 128], F32R, tag="ht")
                    for dc in range(NDC):
                        tp = psT.tile([128, 128], F32R, tag="ps512")
                        nc.tensor.transpose(
                            tp[:], f1[:, dc * 128:(dc + 1) * 128], ident[:])
                        nc.vector.tensor_copy(out=fT[:, dc], in_=tp[:])
                    p2 = psT.tile([128, D], F32, tag="ps512")
                    for dc in range(NDC):
                        nc.tensor.matmul(p2[:], fT[:, dc], w2T_s[:, dc],
                                         start=(dc == 0), stop=(dc == NDC - 1))
                    t2 = pf.tile([128, D], F32, tag="f")
                    nc.vector.tensor_add(t2[:], p2[:], b2_s)
                    r2 = pf.tile([128, D], F32, tag="f")
                    nc.vector.tensor_add(r2[:], t2[:], hid[:].bitcast(F32))
                    o_t = po.tile([128, D], F32)
                    layer_norm(r2[:], g2_s, be2_s, o_t, psm, pf)
                    nc.sync.dma_start(out=out_o[qt * 128:(qt + 1) * 128, :],
                                      in_=o_t[:])
        if timing_reps:
            _loop.__exit__(None, None, None)

    nc.finalize()
    return nc


def make_in_maps(inp, wq, wk, wv, ln1_g, ln1_b, w1, b1, w2, b2, ln2_g, ln2_b):
    scale = 1.0 / np.sqrt(DH)
    wqT = np.ascontiguousarray((wq * scale).T.astype(np.float32))
    wkT = np.ascontiguousarray(wk.T.astype(np.float32))
    wvT = np.ascontiguousarray(wv.T.astype(np.float32))
    w1T = np.ascontiguousarray(w1.T.astype(np.float32))
    w2T = np.ascontiguousarray(w2.T.astype(np.float32))
    vecs = np.stack([
        np.tile(np.asarray(v, np.float32)[None, :], (128, 1))
        for v in (b1, b2, ln1_g, ln1_b, ln2_g, ln2_b)
    ], axis=1)  # [128, 6, D]
    idn = np.eye(128, dtype=np.float32)

    in_maps = []
    for c in range(N_CORES):
        b = c // 2
        ql = (c % 2) * LQ
        ib = np.asarray(inp[b], np.float32)
        ibT = np.ascontiguousarray(ib.T)
        in_maps.append({
            "inpT": ibT,
            "inpTq": np.ascontiguousarray(ibT[:, ql:ql + LQ]),
            "inpN": np.ascontiguousarray(ib[ql:ql + LQ]),
            "wqT": wqT, "wkT": wkT, "wvT": wvT, "w1T": w1T, "w2T": w2T,
            "vecs": vecs, "idn": idn,
        })
    return in_maps


def kernel(inp, wq, wk, wv, ln1_g, ln1_b, w1, b1, w2, b2, ln2_g, ln2_b):
    if "nc" not in _CACHE:
        _CACHE["nc"] = _build_program()
    nc = _CACHE["nc"]
    in_maps = make_in_maps(inp, wq, wk, wv, ln1_g, ln1_b, w1, b1, w2, b2,
                           ln2_g, ln2_b)
    res = run_bass_kernel_spmd(nc, in_maps, core_ids=list(range(N_CORES)))

    out = np.empty((B, L, D), np.float32)
    attn = np.empty((B, H, L, L), np.float32)
    for c in range(N_CORES):
        b = c // 2
        ql = (c % 2) * LQ
        out[b, ql:ql + LQ] = res.results[c]["out_o"]
        attn[b, :, ql:ql + LQ, :] = res.results[c]["attn_o"]
    return (out, attn)


# revision 7
# speedup vs baseline: 42096.3187x; 1.2962x over previous
"""Trainium2 Bass kernel for a dense transformer attention layer.

Reference computation (per batch b):
    q,k,v = inp @ w{q,k,v}.T split into 8 heads of 64
    attn  = softmax(q k^T / 8)            [B, H, L, L]  (output 1)
    ctx   = attn @ v
    hid   = LN(ctx + inp)
    out   = LN(relu(hid@w1.T+b1)@w2.T + b2 + hid)       (output 0)

Sharding: 8 cores, core c handles batch c//2 and query rows
[(c%2)*1024, (c%2)*1024+1024). K/V are computed per-core over the full
sequence (replicated across the 2 cores of a batch) so there is no
cross-core communication.
"""
import sys

sys.path.insert(0, "/opt/trn_rl_repo")

import numpy as np

import concourse.bass as bass
import concourse.tile as tile
from concourse import bacc, mybir
from concourse.bass_utils import run_bass_kernel_spmd

F32 = mybir.dt.float32
F32R = mybir.dt.float32r
EXP = mybir.ActivationFunctionType.Exp
SQRT = mybir.ActivationFunctionType.Sqrt
SUB = mybir.AluOpType.subtract
MULT = mybir.AluOpType.mult

B, L, D = 4, 2048, 512
H, DH = 8, 64
LQ = 1024            # query rows per core
NQT = LQ // 128      # 8 query tiles
NKC = L // 128       # 16 key chunks
NDC = D // 128       # 4 contraction chunks
N_CORES = 8
EPS = 1e-6

_CACHE = {}


def _build_program(timing_reps=None, variant="full"):
    skip_attn_dma = variant in ("noattndma", "noqk")
    skip_qk = variant == "noqk"
    skip_norm = variant in ("nonorm", "noqk")
    dve_exp = variant == "dveexp"
    norm_cols = 256 if variant == "smallnorm" else L
    nc = bacc.Bacc("TRN2", target_bir_lowering=False, debug=False,
                   num_devices=N_CORES)

    inpT = nc.dram_tensor("inpT", [D, L], F32, kind="ExternalInput").ap()
    inpTq = nc.dram_tensor("inpTq", [D, LQ], F32, kind="ExternalInput").ap()
    inpN = nc.dram_tensor("inpN", [LQ, D], F32, kind="ExternalInput").ap()
    wqT = nc.dram_tensor("wqT", [D, D], F32, kind="ExternalInput").ap()
    wkT = nc.dram_tensor("wkT", [D, D], F32, kind="ExternalInput").ap()
    wvT = nc.dram_tensor("wvT", [D, D], F32, kind="ExternalInput").ap()
    w1T = nc.dram_tensor("w1T", [D, D], F32, kind="ExternalInput").ap()
    w2T = nc.dram_tensor("w2T", [D, D], F32, kind="ExternalInput").ap()
    vecs = nc.dram_tensor("vecs", [128, 6, D], F32, kind="ExternalInput").ap()
    idn = nc.dram_tensor("idn", [128, 128], F32, kind="ExternalInput").ap()

    attn_kind = "Internal" if timing_reps else "ExternalOutput"
    attn_o = nc.dram_tensor("attn_o", [H, LQ, L], F32, kind=attn_kind).ap()
    out_o = nc.dram_tensor("out_o", [LQ, D], F32, kind="ExternalOutput").ap()

    with tile.TileContext(nc) as tc:
        if timing_reps:
            _loop = tc.For_i(0, timing_reps, 1)
            _loop.__enter__()
        with tc.tile_pool(name="pc", bufs=1) as pc, \
             tc.tile_pool(name="psS", bufs=2, space="PSUM") as psS, \
             tc.tile_pool(name="psT", bufs=2, space="PSUM") as psT, \
             tc.tile_pool(name="psC", bufs=1, space="PSUM") as psC:

            # ---- persistent SBUF tiles ----
            QT_s = pc.tile([128, NDC, LQ], F32R)     # Q^T (dh-major, /8 folded)
            KT_s = pc.tile([128, NDC, L], F32R)      # K^T (dh-major)
            V_s = pc.tile([128, NKC, D], F32R)       # V natural, per k-chunk
            ctxn_s = pc.tile([128, NQT, D], F32)     # assembled ctx (natural)
            rec_s = pc.tile([128, H * NQT], F32)     # 1/denominator per (h,qt)
            ident = pc.tile([128, 128], F32R)
            eps_t = pc.tile([128, 1], F32)
            vec_s = pc.tile([128, 6, D], F32)        # b1,b2,g1,be1,g2,be2
            w1T_s = pc.tile([128, NDC, D], F32R)
            w2T_s = pc.tile([128, NDC, D], F32R)

            nc.sync.dma_start(out=ident, in_=idn.bitcast(F32R))
            nc.vector.memset(eps_t[:], EPS)
            nc.sync.dma_start(out=vec_s, in_=vecs)
            nc.sync.dma_start(
                out=w1T_s, in_=w1T.rearrange("(c p) m -> p c m", p=128).bitcast(F32R))
            nc.sync.dma_start(
                out=w2T_s, in_=w2T.rearrange("(c p) m -> p c m", p=128).bitcast(F32R))

            # ================= Phase A: projections =================
            with tc.tile_pool(name="pA", bufs=1) as pA:
                inpT_s = pA.tile([128, NDC, L], F32R)
                inpTq_s = pA.tile([128, NDC, LQ], F32R)
                wqT_s = pA.tile([128, NDC, D], F32R)
                wkT_s = pA.tile([128, NDC, D], F32R)
                wvT_s = pA.tile([128, NDC, D], F32R)

                nc.sync.dma_start(
                    out=inpT_s,
                    in_=inpT.rearrange("(c p) l -> p c l", p=128).bitcast(F32R))
                nc.sync.dma_start(
                    out=inpTq_s,
                    in_=inpTq.rearrange("(c p) l -> p c l", p=128).bitcast(F32R))
                for w_s, w_d in ((wqT_s, wqT), (wkT_s, wkT), (wvT_s, wvT)):
                    nc.sync.dma_start(
                        out=w_s,
                        in_=w_d.rearrange("(c p) m -> p c m", p=128).bitcast(F32R))

                # K^T[c*128:(c+1)*128, n*512:(n+1)*512]
                for c in range(NDC):
                    for n in range(L // 512):
                        ps = psT.tile([128, 512], F32, tag="ps512")
                        for dc in range(NDC):
                            nc.tensor.matmul(
                                ps[:],
                                wkT_s[:, dc, c * 128:(c + 1) * 128],
                                inpT_s[:, dc, n * 512:(n + 1) * 512],
                                start=(dc == 0), stop=(dc == NDC - 1))
                        nc.vector.tensor_copy(
                            out=KT_s[:, c, n * 512:(n + 1) * 512], in_=ps[:])
                # Q^T (only the core's query range)
                for c in range(NDC):
                    for n in range(LQ // 512):
                        ps = psT.tile([128, 512], F32, tag="ps512")
                        for dc in range(NDC):
                            nc.tensor.matmul(
                                ps[:],
                                wqT_s[:, dc, c * 128:(c + 1) * 128],
                                inpTq_s[:, dc, n * 512:(n + 1) * 512],
                                start=(dc == 0), stop=(dc == NDC - 1))
                        nc.vector.tensor_copy(
                            out=QT_s[:, c, n * 512:(n + 1) * 512], in_=ps[:])
                # V natural [l, d] per 128-token chunk
                for lc in range(NKC):
                    ps = psT.tile([128, 512], F32, tag="ps512")
                    for dc in range(NDC):
                        nc.tensor.matmul(
                            ps[:],
                            inpT_s[:, dc, lc * 128:(lc + 1) * 128],
                            wvT_s[:, dc, :],
                            start=(dc == 0), stop=(dc == NDC - 1))
                    nc.vector.tensor_copy(out=V_s[:, lc], in_=ps[:])

            # ================= Phase B: attention =================
            with tc.tile_pool(name="pe", bufs=2) as pe, \
                 tc.tile_pool(name="pa", bufs=3) as pa, \
                 tc.tile_pool(name="pt", bufs=4) as ptp, \
                 tc.tile_pool(name="pd", bufs=4) as pd, \
                 tc.tile_pool(name="pct", bufs=2) as pct:

                for h in range(H):
                    hp = (h % 2) * 64
                    hc = h // 2
                    qt_h = QT_s[hp:hp + 64, hc, :]   # [64, LQ]
                    kt_h = KT_s[hp:hp + 64, hc, :]   # [64, L]

                    ctx_ps = psC.tile([64, LQ], F32, tag="ctx")
                    pend = {}

                    def emit_ctx(kc, g):
                        nc.tensor.matmul(
                            ctx_ps[:, g * 512:(g + 1) * 512],
                            V_s[:, kc, h * DH:(h + 1) * DH],
                            pend.pop((kc, g)),
                            start=(kc == 0), stop=(kc == NKC - 1))

                    for qt in range(NQT):
                        # --- S = Q K^T for this q tile (both 1024-col halves)
                        s_half = []
                        if not skip_qk:
                            for i in range(2):
                                s = psS.tile([128, 1024], F32, tag="s")
                                for j in range(2):
                                    n0 = i * 1024 + j * 512
                                    nc.tensor.matmul(
                                        s[:, j * 512:(j + 1) * 512],
                                        qt_h[:, qt * 128:(qt + 1) * 128],
                                        kt_h[:, n0:n0 + 512],
                                        start=True, stop=True)
                                s_half.append(s)
                        # --- S^T chunks for ctx (2 k-chunks per q tile),
                        #     software-pipelined one k-chunk behind
                        for kc in (2 * qt, 2 * qt + 1):
                            for g in range(2):
                                st = psT.tile([128, 512], F32, tag="ps512")
                                nc.tensor.matmul(
                                    st[:],
                                    kt_h[:, kc * 128:(kc + 1) * 128],
                                    qt_h[:, g * 512:(g + 1) * 512],
                                    start=True, stop=True)
                                pt = ptp.tile([128, 512], F32R)
                                if dve_exp:
                                    nc.vector.tensor_copy(out=pt[:], in_=st[:])
                                else:
                                    nc.scalar.activation(out=pt[:], in_=st[:],
                                                         func=EXP)
                                pend[(kc, g)] = pt
                            if kc > 0:
                                emit_ctx(kc - 1, 0)
                                emit_ctx(kc - 1, 1)
                        if skip_qk:
                            continue
                        # --- exp + row sums + normalize + store attn
                        e_t = pe.tile([128, L], F32)
                        dn = pd.tile([128, 2], F32, tag="dn")
                        for i in range(2):
                            nc.scalar.activation(
                                out=e_t[:, i * 1024:(i + 1) * 1024],
                                in_=s_half[i][:], func=EXP,
                                accum_out=dn[:, i:i + 1])
                        ds = pd.tile([128, 1], F32, tag="ds")
                        nc.vector.tensor_add(ds[:], dn[:, 0:1], dn[:, 1:2])
                        at = pa.tile([128, L], F32)
                        if skip_norm:
                            nc.vector.reciprocal(out=ds[:], in_=ds[:])
                            at = e_t
                        else:
                            nc.gpsimd.normalize_recip(
                                at[:, 0:norm_cols], e_t[:, 0:norm_cols], ds[:])
                        nc.vector.tensor_copy(
                            out=rec_s[:, h * NQT + qt:h * NQT + qt + 1], in_=ds[:])
                        if not skip_attn_dma:
                            nc.sync.dma_start(
                                out=attn_o[h, qt * 128:(qt + 1) * 128, :], in_=at[:])
                    emit_ctx(NKC - 1, 0)
                    emit_ctx(NKC - 1, 1)

                    # --- ctx^T -> ctx natural, scaled by 1/denominator
                    ctxT = pct.tile([64, LQ], F32R)
                    nc.vector.tensor_copy(out=ctxT[:], in_=ctx_ps[:])
                    for i in range(NQT):
                        tp = psT.tile([128, DH], F32R, tag="ps512")
                        nc.tensor.transpose(
                            tp[:], ctxT[:, i * 128:(i + 1) * 128],
                            ident[0:64, 0:64])
                        nc.vector.tensor_scalar_mul(
                            out=ctxn_s[:, i, h * DH:(h + 1) * DH],
                            in0=tp[:].bitcast(F32),
                            scalar1=rec_s[:, h * NQT + i:h * NQT + i + 1])

            # ================= Phase C: residual + LN + FFN =================
            b1_s = vec_s[:, 0, :]
            b2_s = vec_s[:, 1, :]
            g1_s = vec_s[:, 2, :]
            be1_s = vec_s[:, 3, :]
            g2_s = vec_s[:, 4, :]
            be2_s = vec_s[:, 5, :]

            def layer_norm(x_ap, g_ap, b_ap, out_tile, spool, fpool):
                st6 = spool.tile([128, 6], F32, tag="st6")
                nc.vector.bn_stats(out=st6[:], in_=x_ap)
                mv = spool.tile([128, 2], F32, tag="mv")
                nc.vector.bn_aggr(out=mv[:], in_=st6[:])
                sd = spool.tile([128, 1], F32, tag="sd")
                nc.scalar.activation(out=sd[:], in_=mv[:, 1:2], func=SQRT,
                                     bias=eps_t[:], scale=1.0)
                rs = spool.tile([128, 1], F32, tag="rs")
                nc.vector.reciprocal(out=rs[:], in_=sd[:])
                nm = fpool.tile([128, D], F32, tag="f")
                nc.vector.tensor_scalar(out=nm[:], in0=x_ap,
                                        scalar1=mv[:, 0:1], scalar2=rs[:],
                                        op0=SUB, op1=MULT)
                gm = fpool.tile([128, D], F32, tag="f")
                nc.vector.tensor_mul(gm[:], nm[:], g_ap)
                nc.vector.tensor_add(out_tile[:], gm[:], b_ap)

            with tc.tile_pool(name="pf", bufs=6) as pf, \
                 tc.tile_pool(name="ph", bufs=2) as ph, \
                 tc.tile_pool(name="pht", bufs=2) as pht, \
                 tc.tile_pool(name="pq", bufs=2) as pq, \
                 tc.tile_pool(name="psm", bufs=8) as psm, \
                 tc.tile_pool(name="po", bufs=2) as po:
                for qt in range(NQT):
                    inq = pq.tile([128, D], F32)
                    nc.sync.dma_start(out=inq,
                                      in_=inpN[qt * 128:(qt + 1) * 128, :])
                    r1 = pf.tile([128, D], F32, tag="f")
                    nc.vector.tensor_add(r1[:], ctxn_s[:, qt], inq[:])
                    hid = ph.tile([128, D], F32R, tag="hid")
                    layer_norm(r1[:], g1_s, be1_s, hid, psm, pf)

                    hT = pht.tile([128, NDC, 128], F32R, tag="ht")
                    for dc in range(NDC):
                        tp = psT.tile([128, 128], F32R, tag="ps512")
                        nc.tensor.transpose(
                            tp[:], hid[:, dc * 128:(dc + 1) * 128], ident[:])
                        nc.vector.tensor_copy(out=hT[:, dc], in_=tp[:])
                    p1 = psT.tile([128, D], F32, tag="ps512")
                    for dc in range(NDC):
                        nc.tensor.matmul(p1[:], hT[:, dc], w1T_s[:, dc],
                                         start=(dc == 0), stop=(dc == NDC - 1))
                    t1 = pf.tile([128, D], F32, tag="f")
                    nc.vector.tensor_add(t1[:], p1[:], b1_s)
                    f1 = ph.tile([128, D], F32R, tag="hid")
                    nc.vector.tensor_scalar_max(out=f1[:], in0=t1[:], scalar1=0.0)

                    fT = pht.tile([128, NDC, 128], F32R, tag="ht")
                    for dc in range(NDC):
                        tp = psT.tile([128, 128], F32R, tag="ps512")
                        nc.tensor.transpose(
                            tp[:], f1[:, dc * 128:(dc + 1) * 128], ident[:])
                        nc.vector.tensor_copy(out=fT[:, dc], in_=tp[:])
                    p2 = psT.tile([128, D], F32, tag="ps512")
                    for dc in range(NDC):
                        nc.tensor.matmul(p2[:], fT[:, dc], w2T_s[:, dc],
                                         start=(dc == 0), stop=(dc == NDC - 1))
                    t2 = pf.tile([128, D], F32, tag="f")
                    nc.vector.tensor_add(t2[:], p2[:], b2_s)
                    r2 = pf.tile([128, D], F32, tag="f")
                    nc.vector.tensor_add(r2[:], t2[:], hid[:].bitcast(F32))
                    o_t = po.tile([128, D], F32)
                    layer_norm(r2[:], g2_s, be2_s, o_t, psm, pf)
                    nc.sync.dma_start(out=out_o[qt * 128:(qt + 1) * 128, :],
                                      in_=o_t[:])
        if timing_reps:
            _loop.__exit__(None, None, None)

    nc.finalize()
    return nc


def make_in_maps(inp, wq, wk, wv, ln1_g, ln1_b, w1, b1, w2, b2, ln2_g, ln2_b):
    scale = 1.0 / np.sqrt(DH)
    wqT = np.ascontiguousarray((wq * scale).T.astype(np.float32))
    wkT = np.ascontiguousarray(wk.T.astype(np.float32))
    wvT = np.ascontiguousarray(wv.T.astype(np.float32))
    w1T = np.ascontiguousarray(w1.T.astype(np.float32))
    w2T = np.ascontiguousarray(w2.T.astype(np.float32))
    vecs = np.stack([
        np.tile(np.asarray(v, np.float32)[None, :], (128, 1))
        for v in (b1, b2, ln1_g, ln1_b, ln2_g, ln2_b)
    ], axis=1)  # [128, 6, D]
    idn = np.eye(128, dtype=np.float32)

    in_maps = []
    for c in range(N_CORES):
        b = c // 2
        ql = (c % 2) * LQ
        ib = np.asarray(inp[b], np.float32)
        ibT = np.ascontiguousarray(ib.T)
        in_maps.append({
            "inpT": ibT,
            "inpTq": np.ascontiguousarray(ibT[:, ql:ql + LQ]),
            "inpN": np.ascontiguousarray(ib[ql:ql + LQ]),
            "wqT": wqT, "wkT": wkT, "wvT": wvT, "w1T": w1T, "w2T": w2T,
            "vecs": vecs, "idn": idn,
        })
    return in_maps


def kernel(inp, wq, wk, wv, ln1_g, ln1_b, w1, b1, w2, b2, ln2_g, ln2_b):
    if "nc" not in _CACHE:
        _CACHE["nc"] = _build_program()
    nc = _CACHE["nc"]
    in_maps = make_in_maps(inp, wq, wk, wv, ln1_g, ln1_b, w1, b1, w2, b2,
                           ln2_g, ln2_b)
    res = run_bass_kernel_spmd(nc, in_maps, core_ids=list(range(N_CORES)))

    out = np.empty((B, L, D), np.float32)
    attn = np.empty((B, H, L, L), np.float32)
    for c in range(N_CORES):
        b = c // 2
        ql = (c % 2) * LQ
        out[b, ql:ql + LQ] = res.results[c]["out_o"]
        attn[b, :, ql:ql + LQ, :] = res.results[c]["attn_o"]
    return (out, attn)
